# revision 2
# baseline (speedup 1.0000x reference)
import numpy as np
import jax
import jax.numpy as jnp
from ml_dtypes import bfloat16

B = 8192        # graphs
NPG = 39        # nodes per graph
N = B * NPG
NC = 8          # neuron cores
GPC = B // NC   # graphs per core
NEG = 0.2

_DIAG = np.arange(NPG)
_BF = jnp.bfloat16
_F32 = jnp.float32


def _dot(a, b, prefer=_F32):
    return jax.lax.dot_general(
        a.astype(_BF), b.astype(_BF),
        (((a.ndim - 1,), (0,)), ((), ())),
        preferred_element_type=prefer)


def _gat(h_in, A, W, a_s, a_d, b):
    # h_in [G,39,fi]; A [G,39,39] bf16, A[g,d,s] = edge count s->d (incl self
    # loop). Attention logits are bounded (|E| < 5 for this data), so softmax
    # is computed without max-subtraction; the denominator rides along the
    # aggregation matmul as an appended ones-column.
    G, _, fi = h_in.shape
    fo = W.shape[1]
    Wa = jnp.concatenate([W, (W @ a_s)[:, None], (W @ a_d)[:, None]], axis=1)
    H = _dot(h_in.reshape(G * NPG, fi), Wa)          # [G*39, fo+2] f32
    h = H[:, :fo].reshape(G, NPG, fo)
    s = H[:, fo].reshape(G, 1, NPG)                  # source logit, free axis
    d = H[:, fo + 1].reshape(G, NPG, 1)              # dest logit, broadcast
    w = A * jnp.exp(jax.nn.leaky_relu(s + d, NEG))   # [G,d,s] f32
    hw = jnp.concatenate(
        [h, jnp.ones((G, NPG, 1), _F32)], axis=2)    # [G,39,fo+1]
    agg = jax.lax.dot_general(
        w.astype(_BF), hw.astype(_BF),
        (((2,), (1,)), ((0,), (0,))),
        preferred_element_type=_F32)                 # [G,39,fo+1]
    out = agg[..., :fo] / agg[..., fo:]              # denom>0: self-loops
    return jax.nn.relu(out + b)


def _fwd(x, A, params):
    (W1, as1, ad1, b1, W2, as2, ad2, b2, W3, as3, ad3, b3,
     W4, as4, ad4, b4, lw1, lb1, lw2, lb2, lw3, lb3) = params
    G = x.shape[0]
    h1 = _gat(x[..., None], A, W1, as1, ad1, b1)
    h2 = _gat(h1, A, W2, as2, ad2, b2)
    h3 = _gat(h2, A, W3, as3, ad3, b3)
    h4 = _gat(h3, A, W4, as4, ad4, b4)
    f = jnp.concatenate([
        x, h1.reshape(G, -1), h2.reshape(G, -1),
        h3.reshape(G, -1), h4.reshape(G, -1),
        jnp.max(x, axis=1, keepdims=True),
        jnp.max(h1, axis=1), jnp.max(h2, axis=1),
        jnp.max(h3, axis=1), jnp.max(h4, axis=1)], axis=1)   # [G,4560]
    f = jax.nn.relu(_dot(f, lw1) + lb1)
    f = jax.nn.relu(_dot(f, lw2) + lb2)
    return _dot(f, lw3) + lb3


_pmapped = jax.pmap(_fwd, in_axes=(0, 0, None))


def _build_C(edge_index):
    src = np.asarray(edge_index[0], dtype=np.int64)
    dst = np.asarray(edge_index[1], dtype=np.int64)
    g = dst // NPG
    sl = src - g * NPG
    dl = dst - g * NPG
    idx = (g * NPG + dl) * NPG + sl
    C = np.bincount(idx, minlength=B * NPG * NPG).astype(np.float32)
    C = C.reshape(B, NPG, NPG)
    C[:, _DIAG, _DIAG] += 1.0   # self loops on every node
    return C.astype(bfloat16)


def kernel(**inputs):
    x = np.asarray(inputs['x'], np.float32).reshape(B, NPG)
    C = _build_C(inputs['edge_index'])
    pnames = []
    for li in range(1, 5):
        pnames += [f'W{li}', f'as{li}', f'ad{li}', f'b{li}']
    pnames += ['lw1', 'lb1', 'lw2', 'lb2', 'lw3', 'lb3']
    params = tuple(jnp.asarray(np.asarray(inputs[k], np.float32))
                   for k in pnames)
    xs = x.reshape(NC, GPC, NPG)
    Cs = C.reshape(NC, GPC, NPG, NPG)
    out = _pmapped(xs, Cs, params)
    return np.asarray(out).reshape(B, 9).astype(np.float32)


# revision 3
# speedup vs baseline: 1.3315x; 1.3315x over previous
import numpy as np
import jax
import jax.numpy as jnp

B = 8192        # graphs
NPG = 39        # nodes per graph
N = B * NPG
NC = 8          # neuron cores
GPC = B // NC   # real graphs per core
GPAD = 1026     # padded graphs per core (multiple of 3)
PK = GPAD // 3  # packs of 3 graphs -> 117-node block-diag tiles
NEG = 0.2

_BF = jnp.bfloat16
_F32 = jnp.float32


def _dot(a, b):
    return jax.lax.dot_general(
        a.astype(_BF), b.astype(_BF),
        (((a.ndim - 1,), (0,)), ((), ())),
        preferred_element_type=_F32)


def _gat(h_in, A, W, a_s, a_d, b):
    # h_in [GPAD,39,fi]; A [PK,117,117] uint8 block-diag edge counts
    # (incl self loops; zeros off-diagonal mask cross-graph pairs).
    # Attention logits are bounded (|E|<5 for this data) so softmax
    # needs no max-subtraction; the denominator rides along the
    # aggregation matmul as an appended ones-column.
    fi = h_in.shape[2]
    fo = W.shape[1]
    Wa = jnp.concatenate([W, (W @ a_s)[:, None], (W @ a_d)[:, None]], axis=1)
    H = _dot(h_in.reshape(GPAD * NPG, fi), Wa)       # [GPAD*39, fo+2] f32
    h = H[:, :fo].astype(_BF).reshape(PK, 117, fo)
    s = H[:, fo].reshape(PK, 1, 117)                 # source logit row
    d = H[:, fo + 1].reshape(PK, 117, 1)             # dest logit col
    E = jax.nn.leaky_relu(s + d, NEG)                # [PK,117,117] f32
    w = (A.astype(_BF) * jnp.exp(E).astype(_BF))     # block-diag weights
    hw = jnp.concatenate(
        [h, jnp.ones((PK, 117, 1), _BF)], axis=2)    # [PK,117,fo+1]
    agg = jax.lax.dot_general(
        w, hw, (((2,), (1,)), ((0,), (0,))),
        preferred_element_type=_F32)                 # [PK,117,fo+1]
    out = agg[..., :fo] / agg[..., fo:]              # denom>0: self-loops
    return jax.nn.relu(out + b).reshape(GPAD, NPG, fo)


def _fwd(x, A, params):
    (W1, as1, ad1, b1, W2, as2, ad2, b2, W3, as3, ad3, b3,
     W4, as4, ad4, b4, lw1, lb1, lw2, lb2, lw3, lb3) = params
    h1 = _gat(x[..., None], A, W1, as1, ad1, b1)
    h2 = _gat(h1, A, W2, as2, ad2, b2)
    h3 = _gat(h2, A, W3, as3, ad3, b3)
    h4 = _gat(h3, A, W4, as4, ad4, b4)
    f = jnp.concatenate([
        x, h1.reshape(GPAD, -1), h2.reshape(GPAD, -1),
        h3.reshape(GPAD, -1), h4.reshape(GPAD, -1),
        jnp.max(x, axis=1, keepdims=True),
        jnp.max(h1, axis=1), jnp.max(h2, axis=1),
        jnp.max(h3, axis=1), jnp.max(h4, axis=1)], axis=1)   # [GPAD,4560]
    f = jax.nn.relu(_dot(f, lw1) + lb1)
    f = jax.nn.relu(_dot(f, lw2) + lb2)
    return _dot(f, lw3) + lb3


_pmapped = jax.pmap(_fwd, in_axes=(0, 0, None))


def _build_A(edge_index):
    # block-diag packed counts: [NC, PK, 117, 117] uint8, 3 graphs/pack
    src = np.asarray(edge_index[0], dtype=np.int64)
    dst = np.asarray(edge_index[1], dtype=np.int64)
    g = dst // NPG
    core = g // GPC
    gl = g - core * GPC
    p = core * PK + gl // 3
    a = gl - (gl // 3) * 3
    dl = dst - g * NPG
    sl = src - g * NPG
    idx = (p * 117 + a * NPG + dl) * 117 + (a * NPG + sl)
    A = np.bincount(idx, minlength=NC * PK * 117 * 117)
    A = A.astype(np.uint8).reshape(NC, PK, 117, 117)
    di = np.arange(117)
    A[:, :, di, di] += 1   # self loops (pad graphs become identity)
    return A


def kernel(**inputs):
    x = np.asarray(inputs['x'], np.float32).reshape(NC, GPC, NPG)
    xp = np.zeros((NC, GPAD, NPG), np.float32)
    xp[:, :GPC] = x
    A = _build_A(inputs['edge_index'])
    pnames = []
    for li in range(1, 5):
        pnames += [f'W{li}', f'as{li}', f'ad{li}', f'b{li}']
    pnames += ['lw1', 'lb1', 'lw2', 'lb2', 'lw3', 'lb3']
    params = tuple(jnp.asarray(np.asarray(inputs[k], np.float32))
                   for k in pnames)
    out = _pmapped(xp, A, params)
    return np.asarray(out[:, :GPC]).reshape(B, 9).astype(np.float32)


# revision 4
# speedup vs baseline: 3.4571x; 2.5965x over previous
import numpy as np
import jax
import jax.numpy as jnp
from ml_dtypes import bfloat16

B = 8192        # graphs
NPG = 39        # nodes per graph
N = B * NPG
NC = 8          # neuron cores
GPC = B // NC   # real graphs per core
GPAD = 1026    # padded graphs per core (multiple of 3)
PK = GPAD // 3  # packs of 3 graphs -> 117-node block-diag tiles
NEG = 0.2

_BF = jnp.bfloat16
_F32 = jnp.float32


def _dot(a, b):
    return jax.lax.dot_general(
        a.astype(_BF), b.astype(_BF),
        (((a.ndim - 1,), (0,)), ((), ())),
        preferred_element_type=_F32)


def _gat(h_aug, A, Wa, fo):
    # h_aug [GPAD,39,fi+1] with trailing ones column; A [PK,117,117] bf16
    # block-diag edge counts (incl self loops; zeros mask cross-graph).
    # Wa [fi+1, fo+3] columns: [W (bias folded via ones row) | ones col |
    # s col | d col]. Bias folding is exact: sum_s alpha = 1. Attention
    # logits are bounded (|E|<5) so softmax needs no max-subtraction; the
    # denominator is the ones-column riding the aggregation matmul, and
    # relu(denom/denom)=1 regenerates the ones column for the next layer.
    fi1 = h_aug.shape[2]
    H = _dot(h_aug.reshape(GPAD * NPG, fi1), Wa)     # [GPAD*39, fo+3] f32
    hw = H[:, :fo + 1].astype(_BF).reshape(PK, 117, fo + 1)
    s = H[:, fo + 1].reshape(PK, 1, 117)             # source logit row
    d = H[:, fo + 2].reshape(PK, 117, 1)             # dest logit col
    z = s + d                                        # [PK,117,117] f32
    E = jnp.maximum(z, NEG * z)                      # leaky relu, slope<1
    w = A * jnp.exp(E).astype(_BF)                   # block-diag weights
    agg = jax.lax.dot_general(
        w, hw, (((2,), (1,)), ((0,), (0,))),
        preferred_element_type=_F32)                 # [PK,117,fo+1]
    out = jax.nn.relu(agg / agg[..., fo:])           # denom>0: self-loops
    return out.reshape(GPAD, NPG, fo + 1)


def _fwd(x_aug, A, Was):
    Wa1, Wa2, Wa3, Wa4, lw1, lb1, lw2, lb2, lw3, lb3 = Was
    h1 = _gat(x_aug, A, Wa1, 8)
    h2 = _gat(h1, A, Wa2, 64)
    h3 = _gat(h2, A, Wa3, 32)
    h4 = _gat(h3, A, Wa4, 9)
    f = jnp.concatenate([
        x_aug[..., 0], h1[..., :8].reshape(GPAD, -1),
        h2[..., :64].reshape(GPAD, -1), h3[..., :32].reshape(GPAD, -1),
        h4[..., :9].reshape(GPAD, -1),
        jnp.max(x_aug[..., 0], axis=1, keepdims=True),
        jnp.max(h1[..., :8], axis=1), jnp.max(h2[..., :64], axis=1),
        jnp.max(h3[..., :32], axis=1),
        jnp.max(h4[..., :9], axis=1)], axis=1)       # [GPAD,4560]
    f = jax.nn.relu(_dot(f, lw1) + lb1)
    f = jax.nn.relu(_dot(f, lw2) + lb2)
    return _dot(f, lw3) + lb3


_pmapped = jax.pmap(_fwd, in_axes=(0, 0, None))


def _make_wa(W, a_s, a_d, b):
    fi, fo = W.shape
    Wa = np.zeros((fi + 1, fo + 3), np.float32)
    Wa[:fi, :fo] = W
    Wa[fi, :fo] = b           # bias via ones feature
    Wa[fi, fo] = 1.0          # ones column -> softmax denominator
    Wa[:fi, fo + 1] = W @ a_s
    Wa[:fi, fo + 2] = W @ a_d
    return Wa


def _build_A(edge_index):
    # block-diag packed counts: [NC, PK, 117, 117] bf16, 3 graphs/pack
    src = np.asarray(edge_index[0], dtype=np.int64)
    dst = np.asarray(edge_index[1], dtype=np.int64)
    g = dst // NPG
    core = g // GPC
    gl = g - core * GPC
    p = core * PK + gl // 3
    a = gl - (gl // 3) * 3
    dl = dst - g * NPG
    sl = src - g * NPG
    idx = (p * 117 + a * NPG + dl) * 117 + (a * NPG + sl)
    A = np.bincount(idx, minlength=NC * PK * 117 * 117)
    A = A.astype(np.float32).reshape(NC, PK, 117, 117)
    di = np.arange(117)
    A[:, :, di, di] += 1.0   # self loops (pad graphs become identity)
    return A.astype(bfloat16)


def _prep(inputs):
    x = np.asarray(inputs['x'], np.float32).reshape(NC, GPC, NPG)
    xp = np.zeros((NC, GPAD, NPG, 2), np.float32)
    xp[:, :GPC, :, 0] = x
    xp[..., 1] = 1.0
    A = _build_A(inputs['edge_index'])
    Was = []
    for li in range(1, 5):
        Was.append(_make_wa(
            np.asarray(inputs[f'W{li}'], np.float32),
            np.asarray(inputs[f'as{li}'], np.float32),
            np.asarray(inputs[f'ad{li}'], np.float32),
            np.asarray(inputs[f'b{li}'], np.float32)))
    for k in ('lw1', 'lb1', 'lw2', 'lb2', 'lw3', 'lb3'):
        Was.append(np.asarray(inputs[k], np.float32))
    return xp, A, tuple(Was)


def kernel(**inputs):
    xp, A, Was = _prep(inputs)
    out = _pmapped(xp, A, tuple(jnp.asarray(w) for w in Was))
    return np.asarray(out[:, :GPC]).reshape(B, 9).astype(np.float32)


# revision 5
# speedup vs baseline: 4.0815x; 1.1806x over previous
import numpy as np
import jax
import jax.numpy as jnp
from ml_dtypes import bfloat16

B = 8192        # graphs
NPG = 39        # nodes per graph
N = B * NPG
NC = 8          # neuron cores
GPC = B // NC   # real graphs per core
GPAD = 1026     # padded graphs per core (multiple of 3)
PK = GPAD // 3  # packs of 3 graphs -> 117-node block-diag tiles
NEG = 0.2

_BF = jnp.bfloat16
_F32 = jnp.float32


def _dot(a, b):
    return jax.lax.dot_general(
        a.astype(_BF), b.astype(_BF),
        (((a.ndim - 1,), (0,)), ((), ())),
        preferred_element_type=_F32)


def _gat(h_in, A, W, a_s, a_d, b):
    # h_in [GPAD,39,fi]; A [PK,117,117] bf16 block-diag edge counts
    # (incl self loops; zeros off-diagonal mask cross-graph pairs).
    # Attention logits are bounded (|E|<5 for this data) so softmax
    # needs no max-subtraction; the denominator rides along the
    # aggregation matmul as an appended ones-column.
    fi = h_in.shape[2]
    fo = W.shape[1]
    Wa = jnp.concatenate([W, (W @ a_s)[:, None], (W @ a_d)[:, None]], axis=1)
    H = _dot(h_in.reshape(GPAD * NPG, fi), Wa)       # [GPAD*39, fo+2] f32
    h = H[:, :fo].astype(_BF).reshape(PK, 117, fo)
    s = H[:, fo].reshape(PK, 1, 117)                 # source logit row
    d = H[:, fo + 1].reshape(PK, 117, 1)             # dest logit col
    z = s + d                                        # [PK,117,117] f32
    E = jnp.maximum(z, NEG * z)                      # leaky relu, slope<1
    w = A * jnp.exp(E).astype(_BF)                   # block-diag weights
    hw = jnp.concatenate(
        [h, jnp.ones((PK, 117, 1), _BF)], axis=2)    # [PK,117,fo+1]
    agg = jax.lax.dot_general(
        w, hw, (((2,), (1,)), ((0,), (0,))),
        preferred_element_type=_F32)                 # [PK,117,fo+1]
    out = agg[..., :fo] / agg[..., fo:]              # denom>0: self-loops
    return jax.nn.relu(out + b).reshape(GPAD, NPG, fo)


def _fwd(x, A, params):
    (W1, as1, ad1, b1, W2, as2, ad2, b2, W3, as3, ad3, b3,
     W4, as4, ad4, b4, lw1, lb1, lw2, lb2, lw3, lb3) = params
    h1 = _gat(x[..., None], A, W1, as1, ad1, b1)
    h2 = _gat(h1, A, W2, as2, ad2, b2)
    h3 = _gat(h2, A, W3, as3, ad3, b3)
    h4 = _gat(h3, A, W4, as4, ad4, b4)
    f = jnp.concatenate([
        x, h1.reshape(GPAD, -1), h2.reshape(GPAD, -1),
        h3.reshape(GPAD, -1), h4.reshape(GPAD, -1),
        jnp.max(x, axis=1, keepdims=True),
        jnp.max(h1, axis=1), jnp.max(h2, axis=1),
        jnp.max(h3, axis=1), jnp.max(h4, axis=1)], axis=1)   # [GPAD,4560]
    f = jax.nn.relu(_dot(f, lw1) + lb1)
    f = jax.nn.relu(_dot(f, lw2) + lb2)
    return _dot(f, lw3) + lb3


_pmapped = jax.pmap(_fwd, in_axes=(0, 0, None))


def _build_A(edge_index):
    # block-diag packed counts: [NC, PK, 117, 117] bf16, 3 graphs/pack
    src = np.asarray(edge_index[0], dtype=np.int64)
    dst = np.asarray(edge_index[1], dtype=np.int64)
    g = dst // NPG
    core = g // GPC
    gl = g - core * GPC
    p = core * PK + gl // 3
    a = gl - (gl // 3) * 3
    dl = dst - g * NPG
    sl = src - g * NPG
    idx = (p * 117 + a * NPG + dl) * 117 + (a * NPG + sl)
    A = np.bincount(idx, minlength=NC * PK * 117 * 117)
    A = A.astype(np.float32).reshape(NC, PK, 117, 117)
    di = np.arange(117)
    A[:, :, di, di] += 1.0   # self loops (pad graphs become identity)
    return A.astype(bfloat16)


def _prep(inputs):
    x = np.asarray(inputs['x'], np.float32).reshape(NC, GPC, NPG)
    xp = np.zeros((NC, GPAD, NPG), np.float32)
    xp[:, :GPC] = x
    A = _build_A(inputs['edge_index'])
    pnames = []
    for li in range(1, 5):
        pnames += [f'W{li}', f'as{li}', f'ad{li}', f'b{li}']
    pnames += ['lw1', 'lb1', 'lw2', 'lb2', 'lw3', 'lb3']
    params = tuple(np.asarray(inputs[k], np.float32) for k in pnames)
    return xp, A, params


def kernel(**inputs):
    xp, A, params = _prep(inputs)
    out = _pmapped(xp, A, tuple(jnp.asarray(p) for p in params))
    return np.asarray(out[:, :GPC]).reshape(B, 9).astype(np.float32)


# revision 7
# speedup vs baseline: 4.3498x; 1.0657x over previous
import numpy as np
import jax
import jax.numpy as jnp
from ml_dtypes import bfloat16

B = 8192        # graphs
NPG = 39        # nodes per graph
N = B * NPG
NC = 8          # neuron cores
GPC = B // NC   # real graphs per core
GPAD = 1026     # padded graphs per core (multiple of 3)
PK = GPAD // 3  # packs of 3 graphs -> 117-node block-diag tiles
NEG = 0.2

_BF = jnp.bfloat16
_F32 = jnp.float32


def _dot(a, b):
    return jax.lax.dot_general(
        a.astype(_BF), b.astype(_BF),
        (((a.ndim - 1,), (0,)), ((), ())),
        preferred_element_type=_F32)


def _gat(h_in, A, W, a_s, a_d, b):
    # h_in [GPAD,39,fi]; A [PK,117,117] bf16 block-diag edge counts
    # (incl self loops; zeros off-diagonal mask cross-graph pairs).
    # Attention logits are bounded (|E|<5 for this data) so softmax
    # needs no max-subtraction; the denominator rides along the
    # aggregation matmul as an appended ones-column.
    fi = h_in.shape[2]
    fo = W.shape[1]
    Wa = jnp.concatenate([W, (W @ a_s)[:, None], (W @ a_d)[:, None]], axis=1)
    H = _dot(h_in.reshape(GPAD * NPG, fi), Wa)       # [GPAD*39, fo+2] f32
    h = H[:, :fo].astype(_BF).reshape(PK, 117, fo)
    s = H[:, fo].reshape(PK, 117, 1)                 # source logit col
    d = H[:, fo + 1].reshape(PK, 1, 117)             # dest logit row
    z = s + d                                        # [PK,117(s),117(d)]
    E = jnp.maximum(z, NEG * z)                      # leaky relu, slope<1
    w = A * jnp.exp(E).astype(_BF)                   # A[p,s,d] block-diag
    hw = jnp.concatenate(
        [h, jnp.ones((PK, 117, 1), _BF)], axis=2)    # [PK,117(s),fo+1]
    agg = jax.lax.dot_general(
        w, hw, (((1,), (1,)), ((0,), (0,))),
        preferred_element_type=_F32)                 # [PK,117(d),fo+1]
    out = agg[..., :fo] / agg[..., fo:]              # denom>0: self-loops
    return jax.nn.relu(out + b).reshape(GPAD, NPG, fo)


def _fwd(x, A, params):
    (W1, as1, ad1, b1, W2, as2, ad2, b2, W3, as3, ad3, b3,
     W4, as4, ad4, b4, lw1, lb1, lw2, lb2, lw3, lb3) = params
    h1 = _gat(x[..., None], A, W1, as1, ad1, b1)
    h2 = _gat(h1, A, W2, as2, ad2, b2)
    h3 = _gat(h2, A, W3, as3, ad3, b3)
    h4 = _gat(h3, A, W4, as4, ad4, b4)
    f = jnp.concatenate([
        x, h1.reshape(GPAD, -1), h2.reshape(GPAD, -1),
        h3.reshape(GPAD, -1), h4.reshape(GPAD, -1),
        jnp.max(x, axis=1, keepdims=True),
        jnp.max(h1, axis=1), jnp.max(h2, axis=1),
        jnp.max(h3, axis=1), jnp.max(h4, axis=1)], axis=1)   # [GPAD,4560]
    f = jax.nn.relu(_dot(f, lw1) + lb1)
    f = jax.nn.relu(_dot(f, lw2) + lb2)
    return _dot(f, lw3) + lb3


_pmapped = jax.pmap(_fwd, in_axes=(0, 0, None))


def _build_A(edge_index):
    # block-diag packed counts: [NC, PK, 117, 117] bf16, 3 graphs/pack
    src = np.asarray(edge_index[0], dtype=np.int64)
    dst = np.asarray(edge_index[1], dtype=np.int64)
    g = dst // NPG
    core = g // GPC
    gl = g - core * GPC
    p = core * PK + gl // 3
    a = gl - (gl // 3) * 3
    dl = dst - g * NPG
    sl = src - g * NPG
    idx = (p * 117 + a * NPG + sl) * 117 + (a * NPG + dl)
    A = np.bincount(idx, minlength=NC * PK * 117 * 117)
    A = A.astype(np.float32).reshape(NC, PK, 117, 117)
    di = np.arange(117)
    A[:, :, di, di] += 1.0   # self loops (pad graphs become identity)
    return A.astype(bfloat16)


def _prep(inputs):
    x = np.asarray(inputs['x'], np.float32).reshape(NC, GPC, NPG)
    xp = np.zeros((NC, GPAD, NPG), np.float32)
    xp[:, :GPC] = x
    A = _build_A(inputs['edge_index'])
    pnames = []
    for li in range(1, 5):
        pnames += [f'W{li}', f'as{li}', f'ad{li}', f'b{li}']
    pnames += ['lw1', 'lb1', 'lw2', 'lb2', 'lw3', 'lb3']
    params = tuple(np.asarray(inputs[k], np.float32) for k in pnames)
    return xp, A, params


def kernel(**inputs):
    xp, A, params = _prep(inputs)
    out = _pmapped(xp, A, tuple(jnp.asarray(p) for p in params))
    return np.asarray(out[:, :GPC]).reshape(B, 9).astype(np.float32)
